# revision 3
# baseline (speedup 1.0000x reference)
"""Restructured Trainium2 kernel for the Devign GatedGraphConv problem.

Changes vs baseline:
- AllGather split into 4 node-range pieces == gather buckets; piece p of
  step s+1 is all-gathered as soon as phase A has produced m for those
  nodes, so the collective hides under the ring-paced gather phase.
- Segment-sum runs bucket-major (outer loop over buckets, inner over bin
  windows); per-(window,bucket) psum partials are accumulated into the
  bf16 agg SBUF tile (scalar copy for bucket 0, vector add after).
- GRU (phase C) is emitted per 480-node window right after that window's
  last-bucket partial lands, and phase A of the next step is emitted per
  piece right after the covering GRU windows — so tensor work overlaps
  the ring and the next step's AG0 fires with minimal exposure.
- One shared PSUM pool: wlo+whi (1 bank x2 bufs each) for segsum windows,
  pc (2 banks x2 bufs) shared by GRU passes and phase-A matmuls = 8 banks.
"""


import math
import contextlib
import numpy as np

# ---------------------------------------------------------------- config ----


class Cfg:
    def __init__(self):
        self.N = 100000
        self.E = 400000
        self.NC = 8
        self.NSH = 12544                 # nodes per core (98 chunks of 128)
        self.NPAD = self.NSH * self.NC
        self.IN_DIM = 100
        self.D = 200
        self.DP = 256                    # padded gather row (DP elems)
        self.MSG_FP8 = True              # messages in fp8e4m3 (256B rows)
        self.G = 600
        self.STEPS = 4
        self.BIN = 96                    # one-hot width / psum bin
        self.NB = math.ceil(self.NSH / self.BIN)          # 131 bins
        self.PACK = 5                    # bins per psum window (480 dsts)
        self.NW = math.ceil(self.NB / self.PACK)          # 27 windows
        self.GW = 3                      # psum windows per gather group
        self.NCH = 240                   # GRU chunk (2 per 480-node window)
        # pieces: 128-aligned quarters of NSH
        self.PSZ = [3072, 3200, 3072, 3200]
        self.POFF = np.concatenate([[0], np.cumsum(self.PSZ)]).astype(int)
        assert self.POFF[-1] == self.NSH
        self.NP = 4


# ---------------------------------------------------------- host planning ----


class Plan:
    pass


def build_plan(cfg: Cfg, edge_index: np.ndarray):
    src = edge_index[0].astype(np.int64)
    dst = edge_index[1].astype(np.int64)
    NB, NK, NCC, BIN = cfg.NB, cfg.NP, cfg.NC, cfg.BIN
    poff = cfg.POFF

    ecore = dst // cfg.NSH
    dstloc = dst % cfg.NSH
    ebin = dstloc // BIN
    cs = src // cfg.NSH
    sl = src % cfg.NSH
    ebuck = np.searchsorted(poff, sl, side="right") - 1      # piece of src
    # row index within bucket k's m_full piece [8, PSZ[k], DP]
    erow = cs * np.asarray(cfg.PSZ)[ebuck] + (sl - poff[ebuck])

    # counts[c, k, b] ; chunks per (k, b) = max over cores
    counts = np.zeros((NCC, NK, NB), np.int64)
    np.add.at(counts, (ecore, ebuck, ebin), 1)
    kchunks = np.ceil(counts.max(axis=0) / 128).astype(np.int64)  # [NK, NB]
    kchunks[kchunks == 0] = 1            # keep psum accum well-defined

    # bucket-major slot layout: bucket k holds bins in order, each
    # kchunks[k,b]*128 slots. Chunk ids also bucket-major.
    Lk = (kchunks.sum(axis=1) * 128).astype(np.int64)
    Sk = np.zeros(NK + 1, np.int64)
    np.cumsum(Lk, out=Sk[1:])
    off = np.zeros((NK, NB), np.int64)
    for k in range(NK):
        off[k, 1:] = np.cumsum(kchunks[k, :-1] * 128)
    TOT = int(Sk[-1])
    assert TOT % 128 == 0

    cb_base = np.zeros((NK, NB), np.int64)   # first chunk id of (k, b)
    run = 0
    chunk_tab = []                            # (k, b, slot_start)
    for k in range(NK):
        for b in range(NB):
            cb_base[k, b] = run
            for j in range(int(kchunks[k, b])):
                chunk_tab.append((k, b, int(Sk[k] + off[k, b] + j * 128)))
                run += 1
    TOTCH = run

    idx_all = np.zeros((NCC, TOT), np.int16)
    dstoff_all = np.full((NCC, 128, TOTCH), -1.0, np.float32)

    order = np.lexsort((dstloc, ebin, ebuck, ecore))
    ro, do_, co, bo, ko = (erow[order], dstloc[order], ecore[order],
                           ebin[order], ebuck[order])
    key = (co * NK + ko) * NB + bo
    bounds = np.flatnonzero(np.r_[True, key[1:] != key[:-1], True])
    for i0, i1 in zip(bounds[:-1], bounds[1:]):
        c, k, b = int(co[i0]), int(ko[i0]), int(bo[i0])
        n = i1 - i0
        base_slot = int(Sk[k] + off[k, b])
        idx_all[c, base_slot:base_slot + n] = ro[i0:i1].astype(np.int16)
        e = np.arange(n)
        cbs = cb_base[k, b] + e // 128
        dstoff_all[c, e % 128, cbs] = (do_[i0:i1] - b * BIN).astype(np.float32)

    idx_wrapped = np.zeros((NCC, 128, TOT // 16), np.int16)
    for c in range(NCC):
        w = idx_all[c].reshape(TOT // 16, 16).T
        idx_wrapped[c] = np.tile(w, (8, 1))

    # gather groups: per bucket, GW psum windows each
    NWIN = cfg.NW
    groups = []                      # list over (k, wg)
    for k in range(NK):
        for w0 in range(0, NWIN, cfg.GW):
            w1 = min(w0 + cfg.GW, NWIN)
            b0 = w0 * cfg.PACK
            b1 = min(w1 * cfg.PACK, NB)
            cb0 = int(cb_base[k, b0])
            cb1 = int(cb_base[k, b1 - 1] + kchunks[k, b1 - 1])
            s0 = int(Sk[k] + off[k, b0])
            s1 = int(Sk[k] + off[k, b1 - 1] + kchunks[k, b1 - 1] * 128)
            wins = []
            for w in range(w0, w1):
                wb0 = w * cfg.PACK
                wb1 = min(wb0 + cfg.PACK, NB)
                wins.append(dict(w=w, b0=wb0, b1=wb1))
            groups.append(dict(k=k, w0=w0, w1=w1, cbs=(cb0, cb1),
                               srange=(s0, s1), wins=wins))

    p = Plan()
    p.kchunks, p.Sk, p.off, p.cb_base = kchunks, Sk, off, cb_base
    p.TOT, p.TOTCH = TOT, TOTCH
    p.chunk_tab = chunk_tab
    p.groups = groups
    p.idx_wrapped = idx_wrapped
    p.dstoff = dstoff_all
    return p


def host_inputs(cfg: Cfg, plan, inputs):
    import ml_dtypes
    bf16 = ml_dtypes.bfloat16
    x = np.asarray(inputs["x"], np.float32)
    W = np.asarray(inputs["W"], np.float32)
    w_ih = np.asarray(inputs["w_ih"], np.float32)
    w_hh = np.asarray(inputs["w_hh"], np.float32)
    b_ih = np.asarray(inputs["b_ih"], np.float32)
    b_hh = np.asarray(inputs["b_hh"], np.float32)

    D, G, S = cfg.D, cfg.G, cfg.STEPS
    W_rhs = np.zeros((S, 2, 128, D), np.float32)
    W_rhs[:, 0] = W[:, 0:128, :]
    W_rhs[:, 1, 0:D - 128] = W[:, 128:D, :]

    def lhsT(wmat, bias):
        out = np.zeros((2, 128, G), np.float32)
        wT = wmat.T
        out[0] = wT[0:128]
        out[1, 0:D - 128] = wT[128:D]
        out[1, 96] = bias
        return out

    shared = {
        "w_rhs": W_rhs.astype(bf16),
        "wih": lhsT(w_ih, b_ih).astype(bf16),
        "whh": lhsT(w_hh, b_hh).astype(bf16),
    }

    in_maps = []
    for c in range(cfg.NC):
        lo = c * cfg.NSH
        hi = min((c + 1) * cfg.NSH, cfg.N)
        nreal = max(0, hi - lo)
        xT = np.zeros((128, cfg.NSH), np.float32)
        if nreal > 0:
            xT[0:cfg.IN_DIM, 0:nreal] = x[lo:hi].T
        mask = np.zeros((128, cfg.NSH), np.float32)
        mask[:, 0:nreal] = 1.0
        m = dict(shared)
        m["xT"] = xT.astype(bf16)
        m["mask"] = mask.astype(bf16)
        m["idx"] = plan.idx_wrapped[c]
        st = (plan.dstoff[c][:, :, None] ==
              np.arange(cfg.BIN, dtype=np.float32)[None, None, :])
        m["st"] = st.astype(bf16)
        in_maps.append(m)
    return in_maps


# ------------------------------------------------------------ device build ----


def build_program(cfg: Cfg, plan, timing_mode=False, standin8=False):
    import concourse.bass as bass
    import concourse.bacc as bacc
    import concourse.tile as tile
    import concourse.mybir as mybir
    dt = mybir.dt
    AF = mybir.ActivationFunctionType
    ALU = mybir.AluOpType

    NSH, D, G, DP, BIN = cfg.NSH, cfg.D, cfg.G, cfg.DP, cfg.BIN
    DH = D - 128
    ONE = 96                  # ones row in hi group for bias folding
    KH = ONE + 1
    NCH = cfg.NCH
    WINN = cfg.PACK * BIN     # nodes per GRU window (480)
    NWIN = cfg.NW

    mdt = dt.float8e4 if cfg.MSG_FP8 else dt.bfloat16
    nc = bacc.Bacc("TRN2", target_bir_lowering=False, debug=False,
                   num_devices=1 if timing_mode else cfg.NC)

    xT_in = nc.dram_tensor("xT", [128, NSH], dt.bfloat16, kind="ExternalInput")
    mask_in = nc.dram_tensor("mask", [128, NSH], dt.bfloat16,
                             kind="ExternalInput")
    idx_in = nc.dram_tensor("idx", [128, plan.TOT // 16], dt.int16,
                            kind="ExternalInput")
    st_in = nc.dram_tensor("st", [128, plan.TOTCH, BIN], dt.bfloat16,
                           kind="ExternalInput")
    wrhs_in = nc.dram_tensor("w_rhs", [cfg.STEPS, 2, 128, D], dt.bfloat16,
                             kind="ExternalInput")
    wih_in = nc.dram_tensor("wih", [2, 128, G], dt.bfloat16,
                            kind="ExternalInput")
    whh_in = nc.dram_tensor("whh", [2, 128, G], dt.bfloat16,
                            kind="ExternalInput")
    y_out = nc.dram_tensor("y", [128, 2], dt.float32, kind="ExternalOutput")

    # GRU windows: (n0, n1) node ranges of 480 (last ragged)
    gwin = [(w * WINN, min((w + 1) * WINN, NSH)) for w in range(NWIN)]
    # piece p is ready for phase A once GRU windows covering
    # [POFF[p], POFF[p+1]) are done
    pw_need = [int(math.ceil(cfg.POFF[p + 1] / WINN)) - 1
               for p in range(cfg.NP)]

    with tile.TileContext(nc) as tc:
        with tc.tile_pool(name="persist", bufs=1) as pp, \
             tc.tile_pool(name="dram", bufs=1, space="DRAM") as dram:
            h_lo = pp.tile([128, NSH], dt.bfloat16, tag="h_lo")
            h_hi = pp.tile([128, NSH], dt.bfloat16, tag="h_hi")
            agg_lo = pp.tile([128, cfg.NB * BIN], dt.bfloat16, tag="agg_lo")
            agg_hi = pp.tile([128, cfg.NB * BIN], dt.bfloat16, tag="agg_hi")
            idx_s = pp.tile([128, plan.TOT // 16], dt.int16, tag="idx")
            mask_s = pp.tile([128, NSH], dt.bfloat16, tag="mask")
            y_acc = pp.tile([128, 2], dt.float32, tag="y_acc")
            wrhs_s = pp.tile([128, cfg.STEPS, 2, D], dt.bfloat16, tag="wrhs")
            wih_s = pp.tile([128, 2, G], dt.bfloat16, tag="wih")
            whh_s = pp.tile([128, 2, G], dt.bfloat16, tag="whh")

            m_mine = dram.tile([NSH, DP], mdt, tag="m_mine",
                               name="m_mine")
            # one all-gather destination per (step, piece): single writer
            m_fulls = [[dram.tile([cfg.NC, cfg.PSZ[p], DP], mdt,
                                  addr_space="Local" if (timing_mode or
                                                         standin8)
                                  else "Shared",
                                  tag=f"m_full{s}_{p}",
                                  name=f"m_full{s}_{p}")
                        for p in range(cfg.NP)]
                       for s in range(cfg.STEPS)]

            nc.sync.dma_start(h_lo[:], xT_in.ap())
            nc.sync.dma_start(mask_s[:], mask_in.ap())
            nc.vector.memset(y_acc[:], 0.0)
            nc.sync.dma_start(idx_s[:], idx_in.ap())
            nc.sync.dma_start(wrhs_s[:],
                              wrhs_in.ap().rearrange("s g p m -> p s g m"))
            nc.sync.dma_start(wih_s[:],
                              wih_in.ap().rearrange("g p m -> p g m"))
            nc.sync.dma_start(whh_s[:],
                              whh_in.ap().rearrange("g p m -> p g m"))
            nc.vector.memset(h_hi[:], 0.0)
            nc.vector.memset(agg_hi[:], 0.0)
            nc.vector.memset(h_hi[ONE:ONE + 1, :], 1.0)
            nc.vector.memset(agg_hi[ONE:ONE + 1, :], 1.0)

            phase_stack = contextlib.ExitStack()
            ps = phase_stack.enter_context(
                tc.tile_pool(name="ps", bufs=2, space="PSUM"))
            sbA = phase_stack.enter_context(tc.tile_pool(name="sbA", bufs=2))
            msgB = phase_stack.enter_context(tc.tile_pool(name="msgB",
                                                          bufs=4))
            stB = phase_stack.enter_context(tc.tile_pool(name="stB", bufs=3))
            sbC = phase_stack.enter_context(tc.tile_pool(name="sbC", bufs=2))

            def emit_A_piece(step, p):
                """m[n] = h[n] @ W[step] for nodes of piece p; write to
                m_mine (the all-gather is emitted separately)."""
                n0, n1 = int(cfg.POFF[p]), int(cfg.POFF[p + 1])
                c0, c1 = n0 // 128, n1 // 128
                for g0 in range(c0, c1, 8):
                    g1 = min(g0 + 8, c1)
                    msb = sbA.tile([128, 8, D], mdt, tag="msb")
                    for cp in range(g0, g1, 2):
                        npair = min(2, g1 - cp)
                        pm = ps.tile([128, 2, 2, 256], dt.float32, tag="pc")
                        for j in range(npair):
                            sl = slice((cp + j) * 128, (cp + j + 1) * 128)
                            nc.tensor.matmul(pm[:, j, 0, 0:D], h_lo[:, sl],
                                             wrhs_s[:, step, 0, :],
                                             start=True, stop=False)
                            nc.tensor.matmul(pm[:, j, 0, 0:D],
                                             h_hi[0:DH, sl],
                                             wrhs_s[0:DH, step, 1, :],
                                             start=False, stop=True)
                        nc.scalar.activation(
                            msb[:, cp - g0:cp - g0 + npair, :],
                            pm[:, 0:npair, 0, 0:D], AF.Copy)
                    ngrp = g1 - g0
                    nc.sync.dma_start(
                        m_mine[g0 * 128:g1 * 128, 0:D].rearrange(
                            "(c p) d -> p c d", p=128),
                        msb[:, 0:ngrp, :])
            def emit_AG_piece(step, p):
                n0, n1 = int(cfg.POFF[p]), int(cfg.POFF[p + 1])
                m_full = m_fulls[step][p]
                if timing_mode or standin8:
                    for r in range(cfg.NC):
                        nc.sync.dma_start(m_full[r], m_mine[n0:n1, :])
                else:
                    nc.gpsimd.collective_compute(
                        "AllGather", ALU.bypass,
                        replica_groups=[list(range(cfg.NC))],
                        ins=[m_mine[n0:n1, :].opt()],
                        outs=[m_full.opt()])

            def emit_C_window(w, final=False):
                """GRU update for nodes of window w (after agg complete)."""
                n0w, n1w = gwin[w]
                for ch0 in range(n0w, n1w, NCH):
                    ch1 = min(ch0 + NCH, n1w)
                    wdt = ch1 - ch0
                    nsl = slice(ch0, ch1)

                    def mm_into(pt, j, hv, col0, part, wgt, rhs_lo, rhs_hi,
                                first, last):
                        mm = pt[0:part, j, hv, 0:wdt]
                        cs = slice(col0, col0 + part)
                        nc.tensor.matmul(mm, wgt[:, 0, cs], rhs_lo[:, nsl],
                                         start=first, stop=False)
                        nc.tensor.matmul(mm, wgt[0:KH, 1, cs],
                                         rhs_hi[0:KH, nsl],
                                         start=False, stop=last)

                    p1 = ps.tile([128, 2, 2, 256], dt.float32, tag="pc")
                    for j, col0 in ((0, 0), (1, D)):
                        mm_into(p1, j, 0, col0, 128, wih_s, agg_lo, agg_hi,
                                True, False)
                        mm_into(p1, j, 0, col0, 128, whh_s, h_lo, h_hi,
                                False, True)
                        mm_into(p1, j, 1, col0 + 128, DH, wih_s, agg_lo,
                                agg_hi, True, False)
                        mm_into(p1, j, 1, col0 + 128, DH, whh_s, h_lo, h_hi,
                                False, True)

                    rt_lo = sbC.tile([128, NCH], dt.float32, tag="rt_lo")
                    rt_hi = sbC.tile([128, NCH], dt.float32, tag="rt_hi")
                    zt_lo = sbC.tile([128, NCH], dt.float32, tag="zt_lo")
                    zt_hi = sbC.tile([128, NCH], dt.float32, tag="zt_hi")
                    nc.scalar.activation(rt_lo[:, 0:wdt], p1[:, 0, 0, 0:wdt],
                                         AF.Sigmoid)
                    nc.scalar.activation(rt_hi[0:DH, 0:wdt],
                                         p1[0:DH, 0, 1, 0:wdt], AF.Sigmoid)
                    nc.scalar.activation(zt_lo[:, 0:wdt], p1[:, 1, 0, 0:wdt],
                                         AF.Sigmoid)
                    nc.scalar.activation(zt_hi[0:DH, 0:wdt],
                                         p1[0:DH, 1, 1, 0:wdt], AF.Sigmoid)

                    p2 = ps.tile([128, 2, 2, 256], dt.float32, tag="pc")
                    mm_into(p2, 0, 0, 2 * D, 128, wih_s, agg_lo, agg_hi,
                            True, True)
                    mm_into(p2, 0, 1, 2 * D + 128, DH, wih_s, agg_lo,
                            agg_hi, True, True)
                    mm_into(p2, 1, 0, 2 * D, 128, whh_s, h_lo, h_hi,
                            True, True)
                    mm_into(p2, 1, 1, 2 * D + 128, DH, whh_s, h_lo, h_hi,
                            True, True)

                    t_lo = sbC.tile([128, NCH], dt.float32, tag="t_lo")
                    t_hi = sbC.tile([128, NCH], dt.float32, tag="t_hi")
                    n_lo = sbC.tile([128, NCH], dt.float32, tag="n_lo")
                    n_hi = sbC.tile([128, NCH], dt.float32, tag="n_hi")

                    for (rt, zt, hv, tt, nn, hh, part) in (
                        (rt_lo, zt_lo, 0, t_lo, n_lo, h_lo, 128),
                        (rt_hi, zt_hi, 1, t_hi, n_hi, h_hi, DH),
                    ):
                        pss = slice(0, part)
                        ws = slice(0, wdt)
                        nc.vector.tensor_mul(tt[pss, ws], rt[pss, ws],
                                             p2[pss, 1, hv, ws])
                        nc.vector.tensor_add(tt[pss, ws], tt[pss, ws],
                                             p2[pss, 0, hv, ws])
                        nc.scalar.activation(nn[pss, ws], tt[pss, ws],
                                             AF.Tanh)
                        nc.vector.tensor_sub(tt[pss, ws], hh[pss, nsl],
                                             nn[pss, ws])
                        nc.vector.tensor_mul(tt[pss, ws], tt[pss, ws],
                                             zt[pss, ws])
                        nc.vector.tensor_add(hh[pss, nsl], nn[pss, ws],
                                             tt[pss, ws])
                if final:
                    for col, hh in ((0, h_lo), (1, h_hi)):
                        rl = sbC.tile([128, 512], dt.bfloat16, tag="rl")
                        wn = n1w - n0w
                        nc.scalar.activation(rl[:, 0:wn], hh[:, n0w:n1w],
                                             AF.Relu)
                        nc.vector.tensor_mul(rl[:, 0:wn], rl[:, 0:wn],
                                             mask_s[:, n0w:n1w])
                        yp = sbC.tile([128, 1], dt.float32, tag="yp")
                        nc.vector.reduce_max(yp[:], rl[:, 0:wn],
                                             axis=mybir.AxisListType.X)
                        nc.vector.tensor_max(y_acc[:, col:col + 1],
                                             y_acc[:, col:col + 1], yp[:])

            # ---------------- prologue: A(0) + AG(0) all pieces ------------
            for p in range(cfg.NP):
                emit_A_piece(0, p)
                emit_AG_piece(0, p)

            for step in range(cfg.STEPS):
                # phase B: bucket-major gather + segsum; C/A/AG interleaved
                emitted_w = 0          # GRU windows emitted (this step)
                emitted_p = 0          # next-step A pieces emitted
                emitted_ag = 0         # next-step AG pieces emitted
                step_groups = plan.groups
                if step == cfg.STEPS - 1:
                    # half-range rounds: C windows complete twice as early,
                    # halving the un-overlapped GRU/final tail
                    per_k = {}
                    for g in plan.groups:
                        per_k.setdefault(g["k"], []).append(g)
                    half = (len(per_k[0]) + 1) // 2
                    step_groups = []
                    for rnd in range(2):
                        for k in sorted(per_k):
                            gs = per_k[k]
                            step_groups.extend(
                                gs[:half] if rnd == 0 else gs[half:])
                for grp in step_groups:
                    k = grp["k"]
                    cb0, cb1 = grp["cbs"]
                    s0, s1 = grp["srange"]
                    nch_g = cb1 - cb0
                    m_full = m_fulls[step][k]
                    m_flat = m_full.rearrange("c n d -> (c n) d")

                    st_t = stB.tile([128, nch_g, BIN], dt.bfloat16, tag="st")
                    nc.sync.dma_start(st_t[:], st_in.ap()[:, cb0:cb1, :])
                    mt = msgB.tile([128, nch_g, DP], mdt, tag="msg")
                    nc.gpsimd.dma_gather(
                        out_ap=mt[:],
                        in_ap=m_flat,
                        idxs_ap=idx_s[:, s0 // 16:s1 // 16],
                        num_idxs=s1 - s0,
                        num_idxs_reg=s1 - s0,
                        elem_size=DP,
                        single_packet=False)

                    for win in grp["wins"]:
                        w, b0, b1 = win["w"], win["b0"], win["b1"]
                        npk = b1 - b0
                        plo = ps.tile([128, cfg.PACK, BIN], dt.float32,
                                      tag="wlo")
                        phi = ps.tile([128, cfg.PACK, BIN], dt.float32,
                                      tag="whi")
                        for b in range(b0, b1):
                            nchb = int(plan.kchunks[k, b])
                            cbb = int(plan.cb_base[k, b])
                            for j in range(nchb):
                                lsl = cbb + j - cb0
                                first = j == 0
                                last = j == nchb - 1
                                nc.tensor.matmul(
                                    plo[:, b - b0, :], mt[:, lsl, 0:128],
                                    st_t[:, lsl, :],
                                    start=first, stop=last)
                                nc.tensor.matmul(
                                    phi[0:DH, b - b0, :], mt[:, lsl, 128:D],
                                    st_t[:, lsl, :],
                                    start=first, stop=last)
                        bs = slice(b0 * BIN, b1 * BIN)
                        for (agg, pt, part) in ((agg_lo, plo, 128),
                                                (agg_hi, phi, DH)):
                            av = agg[0:part, bs].rearrange(
                                "p (c x) -> p c x", x=BIN)
                            if k == 0:
                                nc.scalar.activation(
                                    av, pt[0:part, 0:npk, :], AF.Copy)
                            else:
                                nc.vector.tensor_add(
                                    av, av, pt[0:part, 0:npk, :])
                        # after last bucket's partial: GRU for window w
                        if k == cfg.NP - 1:
                            emit_C_window(w, final=step == cfg.STEPS - 1)
                            emitted_w = w + 1
                            # emit next-step A pieces as coverage completes
                            if step < cfg.STEPS - 1:
                                while (emitted_p < cfg.NP and
                                       emitted_w > pw_need[emitted_p]):
                                    emit_A_piece(step + 1, emitted_p)
                                    emitted_p += 1
                    # early AG insertion: one gather group of slack after
                    # the piece's A chain was emitted
                    if (k == cfg.NP - 1 and step < cfg.STEPS - 1):
                        while (emitted_ag < emitted_p and
                               grp["w1"] * cfg.PACK * cfg.BIN >=
                               int(cfg.POFF[emitted_ag + 1]) + 2 * cfg.GW *
                               cfg.PACK * cfg.BIN):
                            emit_AG_piece(step + 1, emitted_ag)
                            emitted_ag += 1
                if step < cfg.STEPS - 1:
                    while emitted_p < cfg.NP:
                        emit_A_piece(step + 1, emitted_p)
                        emitted_p += 1
                    while emitted_ag < cfg.NP:
                        emit_AG_piece(step + 1, emitted_ag)
                        emitted_ag += 1

            nc.sync.dma_start(y_out.ap(), y_acc[:])
            phase_stack.close()

    nc.compile()
    return nc


# ------------------------------------------------------------------ driver ----


def postprocess(cfg: Cfg, y_all, cls_w, cls_b):
    DH = cfg.D - 128
    pooled = np.zeros(cfg.D, np.float64)
    ym = np.max(np.stack(y_all), axis=0)
    pooled[0:128] = ym[:, 0]
    pooled[128:cfg.D] = ym[0:DH, 1]
    logits = pooled @ np.asarray(cls_w, np.float64).T + np.asarray(
        cls_b, np.float64)
    e = np.exp(logits - logits.max())
    sm = e / e.sum()
    return sm[None, :].astype(np.float32)


def kernel(**inputs):
    import sys
    if '/opt/trn_rl_repo' not in sys.path:
        sys.path.insert(0, '/opt/trn_rl_repo')
    from concourse import bass_utils
    cfg = Cfg()
    plan = build_plan(cfg, np.asarray(inputs["edge_index"]))
    in_maps = host_inputs(cfg, plan, inputs)
    nc = build_program(cfg, plan)
    res = bass_utils.run_bass_kernel_spmd(nc, in_maps,
                                          core_ids=list(range(cfg.NC)))
    y_all = [res.results[c]["y"] for c in range(cfg.NC)]
    return postprocess(cfg, y_all, inputs["cls_w"], inputs["cls_b"])
